# revision 30
# baseline (speedup 1.0000x reference)
"""Trainium2 Bass kernel for nn_AttentionBlock (GroupNorm + 8-head self-attention
+ projection + residual) on x: [16, 512, 32, 32].

Sharding: data-parallel over batch across 8 NeuronCores (2 batch items/core),
no collectives.

v5-v13 (237us v2 -> ~209-216us measured; fp8-DR-everything (v3: 300us) and
row-tiled S-pairing with 512-wide exps (v4: 267us) measured and reverted):
hw calibration: matmul cost = rhs free-element count at ~2.4GHz warm
(DoubleRow does NOT stream 2/cycle — it only halves pass/instruction count);
ACT exp = (N+352)/1.2 ns (so wide activations win); row-tiled K<=64 matmuls
at different tile_position rows DO overlap (~2x) but pairing two heads needs
12 PSUM banks (S double-buffers + 2 value accumulators) vs the 8 available,
forcing 512-wide exps whose +161ns/instr overhead eats the gain.
  - Sequential heads with [128,1024] exps (ACT stream ~137us busy; PE ~161us
    busy is the pacer: S 256 matmuls (K=64, half-array — irreducible without
    the PSUM-blocked pairing) + value/qkv/proj DR + ~exposed ldweights).
  - qkv / proj in fp8e4 DR: contraction 512 as 2 passes of [128 x 2 sub]
    (cin = kk*256 + j*128 + p); xn/at stored [128, 2, 2, T] fp8; q/k evac
    to bf16 chunks (bf16 S matmuls keep softmax-score precision).
  - value matmul fp8 DR as v2 (ones column -> softmax denominator in PSUM
    row 0); softmax evac reads PSUM directly (no staging copy): recip(row 0)
    -> gpsimd partition_broadcast -> mul into at8 fp8 (DR layout for proj).
  - PSUM: "s" rotation bufs=3 (S tiles AND qkv/proj accumulators, 6 banks)
    + value accumulator (2 banks): the 3-deep rotation gives the S matmuls
    ~1.5 exp-steps of slack so PE/ACT don't ping-pong (v4's failure mode).
  - Software-pipelined emission as v2: per head, S-pair matmuls back-to-back,
    value DR pairs delayed two m-steps, evac carried into the NEXT head's
    stream (carry popped BEFORE stolen work at mp0 so late at8 writes are
    emitted before proj consumers); qkv of the next batch / proj of the
    previous batch stolen into attention m-steps on a fixed pop plan.
"""
import math
import sys

sys.path.insert(0, "/opt/trn_rl_repo")

import numpy as np

import concourse.bass as bass  # noqa: F401  (registers types)
import concourse.tile as tile
from concourse import bacc, mybir
from concourse.bass_utils import run_bass_kernel_spmd

AF = mybir.ActivationFunctionType
ALU = mybir.AluOpType
F32 = mybir.dt.float32
BF16 = mybir.dt.bfloat16
FP8 = mybir.dt.float8e4
DR = mybir.MatmulPerfMode.DoubleRow

B, C, HH, WW = 16, 512, 32, 32
T = HH * WW            # 1024
NH, CH = 8, 64         # heads, head dim
MV = 128               # padded per-head v^T block: ones | 63 zero | 64 v
VOFF = 64              # v cols start at 64 (quadrant-aligned PSUM rows)
G, CPG = 32, 16        # groupnorm groups, channels per group
EPS = 1e-5
NCORES = 8
BPC = B // NCORES      # 2 batch items per core
P = 128
NCC = C // P           # 4 channel chunks
NKK = 2                # DR contraction passes (256 cin each)
NTC = T // P           # 8 t chunks (m steps)
NPAIR = NTC // 2       # 4 double-row pairs
NN = T // 512          # 2 n-chunks of 512
EXPSHIFT = -4.0        # exp(S + EXPSHIFT); cancels in softmax, keeps fp8 range


def _body(ctx, tc, d):
    nc = tc.nc
    sync = nc.sync

    consts = ctx.enter_context(tc.tile_pool(name="consts", bufs=1))
    xp = ctx.enter_context(tc.tile_pool(name="xp", bufs=2))
    xnp = ctx.enter_context(tc.tile_pool(name="xnp", bufs=2))
    qkp = ctx.enter_context(tc.tile_pool(name="qkp", bufs=1))
    vtp = ctx.enter_context(tc.tile_pool(name="vtp", bufs=2))
    apl = ctx.enter_context(tc.tile_pool(name="apl", bufs=1))
    ep = ctx.enter_context(tc.tile_pool(name="ep", bufs=3))
    smp = ctx.enter_context(tc.tile_pool(name="smp", bufs=2))
    rp = ctx.enter_context(tc.tile_pool(name="rp", bufs=2))
    opl = ctx.enter_context(tc.tile_pool(name="opl", bufs=4))
    ps = ctx.enter_context(tc.tile_pool(name="ps", bufs=3, space="PSUM"))

    # ---- x loads for batch 0 first (startup latency), then constants ----
    xb = {}
    for c in range(NCC):
        xt = xp.tile([P, T], F32, tag=f"x{c}", name=f"x_0_{c}")
        sync.dma_start(xt[:], d["x"][0, c * P:(c + 1) * P, :])
        xb[(0, c)] = xt

    aux = consts.tile([P, 20], F32)              # bqk[0:8] bproj[8:12] gns[12:16] gnb[16:20]
    sync.dma_start(aux[:], d["aux"][:])
    gmats = consts.tile([P, NCC, G], F32)
    sync.dma_start(gmats[:], d["gmats"][:])
    ematT = consts.tile([G, NCC, P], F32)
    sync.dma_start(ematT[:], d["ematT"][:])
    w8 = consts.tile([P, NKK, 2, 3 * C], FP8)    # [128, kk, j, 1536]
    sync.dma_start(w8[:], d["w8"][:])
    wp8 = consts.tile([P, NKK, 2, C], FP8)       # [128, kk, j, 512]
    sync.dma_start(wp8[:], d["wp8"][:])
    ebias = consts.tile([P, 1], F32)
    nc.gpsimd.memset(ebias[:], EXPSHIFT)
    # dummy exp: pulls the ACT Exp-table load into the idle startup window
    # (otherwise it lands right before the first real exp, on the critical
    # path)
    junk = consts.tile([1, 1], F32)
    nc.scalar.activation(junk[:], ebias[0:1, 0:1], AF.Exp)

    xn8b, qt, kt, vt, at8b = {}, {}, {}, {}, {}

    # one-time pad init of all vt physical buffers (tag rotation: 2 bufs/tag):
    # col 0 = ones (softmax denominator weights), cols 1:VOFF = zeros.
    for rep in range(2):
        for mp in range(NPAIR):
            vi = vtp.tile([P, NH, 2, MV], FP8, tag=f"vt{mp}",
                          name=f"vtinit_{rep}_{mp}")
            nc.gpsimd.memset(vi[:, :, :, 0:1], 1.0)
            nc.gpsimd.memset(vi[:, :, :, 1:VOFF], 0.0)

    def load_x(bi):
        for c in range(NCC):
            xt = xp.tile([P, T], F32, tag=f"x{c}", name=f"x_{bi}_{c}")
            sync.dma_start(xt[:], d["x"][bi, c * P:(c + 1) * P, :])
            xb[(bi, c)] = xt

    s12b = {}

    def gn_stats_chunk(bi, c):
        # bn_stats halves the DVE read cost vs reduce_sum + squared-STT:
        # s12[:, c] ends as (mean, E[x^2]) per channel-partition.
        if bi not in s12b:
            s12b[bi] = smp.tile([P, NCC, 2], F32, tag="s12", name=f"s12_{bi}")
            s12b[(bi, "bs")] = smp.tile([P, NCC, 2, 6], F32, tag="bs",
                                        name=f"bs_{bi}")
        s12 = s12b[bi]
        bs = s12b[(bi, "bs")]
        for i in range(2):
            nc.vector.bn_stats(bs[:, c, i, :],
                               xb[(bi, c)][:, i * 512:(i + 1) * 512])
        nc.vector.bn_aggr(s12[:, c, :], bs[:, c, :, :])   # (mean, var)
        nc.vector.scalar_tensor_tensor(
            s12[:, c, 1:2], s12[:, c, 0:1], s12[:, c, 0:1], s12[:, c, 1:2],
            op0=ALU.mult, op1=ALU.add)                    # E[x^2]

    def gn_finish(bi):
        s12 = s12b[bi]
        xn8 = xnp.tile([P, NKK, 2, T], FP8, tag="xn8", name=f"xn8_{bi}")
        xn8b[bi] = xn8
        gsum = ps.tile([G, 2], F32, tag="s", name=f"gsum_{bi}")
        for c in range(NCC):
            nc.tensor.matmul(gsum[:], gmats[:, c, :], s12[:, c, :],
                             start=(c == 0), stop=(c == NCC - 1))

        ms = smp.tile([G, 4], F32, tag="ms", name=f"ms_{bi}")  # mu, msq, var+eps, mu^2
        nc.vector.tensor_scalar_mul(ms[:, 0:2], gsum[:], 1.0 / CPG)
        nc.vector.tensor_mul(ms[:, 3:4], ms[:, 0:1], ms[:, 0:1])
        nc.vector.scalar_tensor_tensor(ms[:, 2:3], ms[:, 1:2], EPS, ms[:, 3:4],
                                       op0=ALU.add, op1=ALU.subtract)
        # rstd = rsqrt(var+eps) on DVE: reciprocal seed + 3 Newton steps
        # (keeps ACT's Exp table resident -- no table reloads)
        musd = smp.tile([G, 2], F32, tag="musd", name=f"musd_{bi}")  # mu, rstd
        nc.vector.tensor_copy(musd[:, 0:1], ms[:, 0:1])
        rs = smp.tile([G, 4], F32, tag="rs", name=f"rs_{bi}")
        nc.vector.reciprocal_approx_fast(rs[:, 0:1], ms[:, 2:3])
        nc.vector.tensor_scalar_mul(rs[:, 1:2], ms[:, 2:3], 0.5)
        for _ in range(3):
            nc.vector.tensor_mul(rs[:, 2:3], rs[:, 0:1], rs[:, 0:1])
            nc.vector.tensor_mul(rs[:, 2:3], rs[:, 2:3], rs[:, 1:2])
            nc.vector.tensor_scalar(out=rs[:, 3:4], in0=rs[:, 2:3],
                                    scalar1=-1.0, scalar2=1.5,
                                    op0=ALU.mult, op1=ALU.add)
            nc.vector.tensor_mul(rs[:, 0:1], rs[:, 0:1], rs[:, 3:4])
        nc.vector.tensor_copy(musd[:, 1:2], rs[:, 0:1])

        for c in range(NCC):
            chan = ps.tile([P, 2], F32, tag="s", name=f"chan_{bi}_{c}")
            nc.tensor.matmul(chan[:], ematT[:, c, :], musd[:], start=True, stop=True)
            ac = smp.tile([P, 3], F32, tag=f"aff{c}", name=f"aff_{bi}_{c}")  # a, -a, b
            nc.vector.tensor_mul(ac[:, 0:1], aux[:, 12 + c:13 + c], chan[:, 1:2])
            nc.vector.tensor_scalar_mul(ac[:, 1:2], ac[:, 0:1], -1.0)
            nc.vector.scalar_tensor_tensor(
                ac[:, 2:3], chan[:, 0:1], ac[:, 1:2], aux[:, 16 + c:17 + c],
                op0=ALU.mult, op1=ALU.add)
            if bi == 0 and c >= 2:
                # startup: split affines across DVE and the idle ACT engine
                # (Identity is in the Exp table set -- no table reload).
                # batch 1 keeps all-DVE: ACT is the pacer mid-stream.
                nc.scalar.activation(
                    xn8[:, c // 2, c % 2, :], xb[(bi, c)][:], AF.Identity,
                    bias=ac[:, 2:3], scale=ac[:, 0:1])
            else:
                nc.vector.tensor_scalar(
                    out=xn8[:, c // 2, c % 2, :], in0=xb[(bi, c)][:],
                    scalar1=ac[:, 0:1], scalar2=ac[:, 2:3],
                    op0=ALU.mult, op1=ALU.add)

    def gn(bi):
        for c in range(NCC):
            gn_stats_chunk(bi, c)
        gn_finish(bi)

    def qkv_groups(bi):
        """Emit-closures, one per psum accumulation group."""
        xn8 = xn8b[bi]
        for oc in range(NCC):
            qt[(bi, oc)] = qkp.tile([P, T], BF16, tag=f"q{oc}", bufs=2,
                                    name=f"q_{bi}_{oc}")
            kt[(bi, oc)] = qkp.tile([P, T], BF16, tag=f"k{oc}", bufs=2,
                                    name=f"k_{bi}_{oc}")
        for mp in range(NPAIR):
            vt[(bi, mp)] = vtp.tile([P, NH, 2, MV], FP8, tag=f"vt{mp}",
                                    name=f"vt_{bi}_{mp}")

        def qk_group(dst, base, boff, oc, n):
            # chunk oc (heads 2oc, 2oc+1), fp8 DR over cin, evac -> bf16
            def emit():
                acc = ps.tile([P, 512], F32, tag="s", name=f"qk_{bi}_{base}_{oc}_{n}")
                for kk in range(NKK):
                    nc.tensor.matmul(
                        acc[:], w8[:, kk, :, base + oc * P:base + (oc + 1) * P],
                        xn8[:, kk, :, n * 512:(n + 1) * 512],
                        start=(kk == 0), stop=(kk == NKK - 1), perf_mode=DR)
                nc.vector.tensor_scalar_add(
                    dst[(bi, oc)][:, n * 512:(n + 1) * 512], acc[:],
                    aux[:, boff + oc:boff + oc + 1])
            return emit

        def v_group(mp):
            def emit():
                vtt = vt[(bi, mp)]
                # both jj-halves in ONE rotation slot (2 banks) so steals
                # don't squeeze the S-tile rotation
                acc = ps.tile([P, 2, 512], F32, tag="s", name=f"v_{bi}_{mp}")
                for kk in range(NKK):
                    for jj in range(2):
                        m = 2 * mp + jj
                        nc.tensor.matmul(
                            acc[:, jj, :], xn8[:, kk, :, m * P:(m + 1) * P],
                            w8[:, kk, :, 2 * C:3 * C],
                            start=(kk == 0), stop=(kk == NKK - 1), perf_mode=DR)
                for jj in range(2):
                    nc.vector.tensor_copy(
                        vtt[:, :, jj, VOFF:VOFF + CH],
                        acc[:, jj, :].rearrange("p (h c) -> p h c", c=CH))
            return emit

        # steal order: ALL v pairs first (vpair(2)/(3) of head h run in the
        # h+1.mp0 carry, so vt must be complete within the first head), then
        # q/k chunks 1..3 in head-consumption order (chunk 0 is upfront).
        groups = [v_group(0), v_group(1), v_group(2), v_group(3),
                  qk_group(qt, 0, 0, 1, 0), qk_group(kt, C, 4, 1, 0),
                  qk_group(qt, 0, 0, 1, 1), qk_group(kt, C, 4, 1, 1),
                  qk_group(qt, 0, 0, 2, 0), qk_group(kt, C, 4, 2, 0),
                  qk_group(qt, 0, 0, 2, 1), qk_group(kt, C, 4, 2, 1),
                  qk_group(qt, 0, 0, 3, 0), qk_group(kt, C, 4, 3, 0),
                  qk_group(qt, 0, 0, 3, 1), qk_group(kt, C, 4, 3, 1)]
        upfront = [qk_group(qt, 0, 0, 0, 0), qk_group(kt, C, 4, 0, 0),
                   qk_group(qt, 0, 0, 0, 1), qk_group(kt, C, 4, 0, 1)]
        return upfront, groups

    def alloc_a(bi):
        at8b[bi] = apl.tile([P, NKK, 2, T], FP8, tag="at8", bufs=2,
                            name=f"at8_{bi}")

    def proj_pair(bi, oc):
        at8 = at8b[bi]
        acc = ps.tile([P, 2, 512], F32, tag="s", name=f"pp_{bi}_{oc}")
        for kk in range(NKK):
            for n in range(NN):
                nc.tensor.matmul(acc[:, n, :],
                                 wp8[:, kk, :, oc * P:(oc + 1) * P],
                                 at8[:, kk, :, n * 512:(n + 1) * 512],
                                 start=(kk == 0), stop=(kk == NKK - 1),
                                 perf_mode=DR)
        for n in range(NN):
            ot = opl.tile([P, 512], F32, tag="o", name=f"op_{bi}_{oc}_{n}")
            nc.vector.scalar_tensor_tensor(
                ot[:], acc[:, n, :], aux[:, 8 + oc:9 + oc],
                xb[(bi, oc)][:, n * 512:(n + 1) * 512],
                op0=ALU.add, op1=ALU.add)
            sync.dma_start(d["out"][bi, oc * P:(oc + 1) * P,
                                    n * 512:(n + 1) * 512], ot[:])

    def proj_groups(bi):
        return [(lambda oc=oc: proj_pair(bi, oc)) for oc in range(NCC)]

    def attn_head(bi, h, work, carry, plan=(1, 1, 1, 1)):
        """One head: S^T + exp stream; value DR pairs delayed two m-steps so
        the final pair + evac land in the NEXT head's stream (via `carry`).
        `plan[mp]` work groups are stolen at each m-pair step."""
        po = (h % 2) * CH
        qh = qt[(bi, h // 2)][po:po + CH, :]
        kh = kt[(bi, h // 2)][po:po + CH, :]
        etp = {}
        st = {}

        def vpair(p):
            if "a" not in st:
                st["a"] = ps.tile([MV, T], F32, tag="aacc", bufs=1,
                                  name=f"aacc_{bi}_{h}")
            for n in range(NN):
                nc.tensor.matmul(
                    st["a"][:, n * 512:(n + 1) * 512],
                    vt[(bi, p)][:, h, :, :],
                    etp[p][:, :, n * 512:(n + 1) * 512],
                    start=(p == 0), stop=(p == NPAIR - 1),
                    perf_mode=DR)

        def evac():
            # PSUM row 0 = softmax denominator, rows VOFF:VOFF+64 = raw a.
            # Stage to SBUF fast (frees the single aacc buffer for the next
            # head's value matmuls), then recip -> broadcast -> normalize.
            vpair(NPAIR - 2)
            vpair(NPAIR - 1)
            at8 = at8b[bi]
            final = (bi == 1 and h == NH - 1)
            if final:
                # nothing follows: skip staging, pipeline in n-halves so the
                # tail projs can start on the first half sooner
                rr1 = rp.tile([1, T], F32, tag="rr1", bufs=3,
                              name=f"rr_{bi}_{h}")
                nc.vector.reciprocal_approx_fast(rr1[:], st["a"][0:1, :])
                for n in range(NN):
                    sl = slice(n * 512, (n + 1) * 512)
                    rb = rp.tile([P, 512], F32, tag=f"rbh{n}", bufs=1,
                                 name=f"rb_{bi}_{h}_{n}")
                    nc.gpsimd.partition_broadcast(rb[:], rr1[:, sl])
                    nc.vector.tensor_mul(
                        at8[po:po + CH, h // 4, (h // 2) % 2, sl],
                        st["a"][VOFF:VOFF + CH, sl], rb[VOFF:VOFF + CH, :])
                return
            a96 = rp.tile([MV, T], F32, tag="a96", bufs=3, name=f"a96_{bi}_{h}")
            nc.vector.tensor_copy(a96[:], st["a"][:])
            rr1 = rp.tile([1, T], F32, tag="rr1", bufs=3, name=f"rr_{bi}_{h}")
            nc.vector.reciprocal_approx_fast(rr1[:], a96[0:1, :])
            rb = rp.tile([P, T], F32, tag="rb", bufs=3, name=f"rb_{bi}_{h}")
            nc.gpsimd.partition_broadcast(rb[:], rr1[:])
            nc.vector.tensor_mul(
                at8[po:po + CH, h // 4, (h // 2) % 2, :],
                a96[VOFF:VOFF + CH, :], rb[VOFF:VOFF + CH, :])

        for mp in range(NPAIR):
            etp[mp] = ep.tile([P, 2, T], FP8, tag="et", bufs=8,
                              name=f"e_{bi}_{h}_{mp}")
            sp2 = []
            for j in range(2):
                m = 2 * mp + j
                sps = ps.tile([P, T], F32, tag="s", name=f"s_{bi}_{h}_{m}")
                sp2.append(sps)
                for n in range(NN):
                    nc.tensor.matmul(sps[:, n * 512:(n + 1) * 512],
                                     kh[:, m * P:(m + 1) * P],
                                     qh[:, n * 512:(n + 1) * 512],
                                     start=True, stop=True)
            for j in range(2):
                nc.scalar.activation(etp[mp][:, j, :], sp2[j][:], AF.Exp,
                                     bias=ebias[:])
            if mp == 0:
                while carry:
                    carry.pop(0)()
            for _ in range(plan[mp]):
                if work:
                    work.pop(0)()
            if mp >= 2:
                vpair(mp - 2)
        carry.append(evac)

    # ---------- software-pipelined emission ----------
    gn(0)
    up0, g0 = qkv_groups(0)
    for g in up0:
        g()
    load_x(1)
    alloc_a(0)
    carry = []
    pending = g0
    for h in range(NH):
        attn_head(0, h, pending, carry)
        # batch-1 GN deferred past the startup-critical window (engines run
        # by readiness: early emission steals DVE from batch-0's affines)
        if h in (1, 2):
            gn_stats_chunk(1, 2 * (h - 1))
            gn_stats_chunk(1, 2 * (h - 1) + 1)
        if h == 3:
            gn_finish(1)
            up1, g1 = qkv_groups(1)
            pending += up1 + g1
    alloc_a(1)
    pending += proj_groups(0)
    for h in range(NH):
        attn_head(1, h, pending, carry)
    while carry:
        carry.pop(0)()
    for g in pending:
        g()
    for oc in range(NCC):
        proj_pair(1, oc)


def build():
    from contextlib import ExitStack

    nc = bacc.Bacc("TRN2", target_bir_lowering=False, debug=False,
                   num_devices=NCORES)
    d = {
        "x": nc.dram_tensor("x", [BPC, C, T], F32, kind="ExternalInput").ap(),
        "w8": nc.dram_tensor("w8", [P, NKK, 2, 3 * C], FP8, kind="ExternalInput").ap(),
        "wp8": nc.dram_tensor("wp8", [P, NKK, 2, C], FP8, kind="ExternalInput").ap(),
        "aux": nc.dram_tensor("aux", [P, 20], F32, kind="ExternalInput").ap(),
        "gmats": nc.dram_tensor("gmats", [P, NCC, G], F32, kind="ExternalInput").ap(),
        "ematT": nc.dram_tensor("ematT", [G, NCC, P], F32, kind="ExternalInput").ap(),
        "out": nc.dram_tensor("out", [BPC, C, T], F32, kind="ExternalOutput").ap(),
    }
    with tile.TileContext(nc) as tc:
        with ExitStack() as ctx:
            _body(ctx, tc, d)
    nc.compile()
    return nc


_CACHE = {}


def prep_inputs(x, gn_scale, gn_bias, w_qkv, b_qkv, w_proj, b_proj):
    import ml_dtypes

    x = np.ascontiguousarray(np.asarray(x, np.float32).reshape(B, C, T))
    gn_scale = np.asarray(gn_scale, np.float32)
    gn_bias = np.asarray(gn_bias, np.float32)
    w_qkv = np.asarray(w_qkv, np.float32)
    b_qkv = np.asarray(b_qkv, np.float32)
    w_proj = np.asarray(w_proj, np.float32)
    b_proj = np.asarray(b_proj, np.float32)

    s = 1.0 / math.sqrt(math.sqrt(CH))
    Wall = w_qkv.copy()
    Wall[:2 * C] *= s                        # fold attention scale into q,k
    # DR layout: w8[p, kk, j, o] = Wall[o, kk*256 + j*128 + p]
    w8 = np.ascontiguousarray(
        Wall.T.reshape(NKK, 2, P, 3 * C).transpose(2, 0, 1, 3)
    ).astype(ml_dtypes.float8_e4m3)
    wp8 = np.ascontiguousarray(
        w_proj.T.reshape(NKK, 2, P, C).transpose(2, 0, 1, 3)
    ).astype(ml_dtypes.float8_e4m3)

    bqk = (b_qkv[:2 * C] * s).reshape(2 * NCC, P).T          # [128, 8]
    bproj_eff = (b_proj + w_proj @ b_qkv[2 * C:]).reshape(NCC, P).T  # [128, 4]
    gns = gn_scale.reshape(NCC, P).T
    gnb = gn_bias.reshape(NCC, P).T
    aux = np.ascontiguousarray(
        np.concatenate([bqk, bproj_eff, gns, gnb], axis=1), np.float32)

    p = np.arange(P)
    gmats = np.zeros((P, NCC, G), np.float32)
    ematT = np.zeros((G, NCC, P), np.float32)
    for c in range(NCC):
        gmats[p, c, 8 * c + p // CPG] = 1.0
        ematT[8 * c + p // CPG, c, p] = 1.0

    shared = {"w8": w8, "wp8": wp8, "aux": aux, "gmats": gmats, "ematT": ematT}
    in_maps = []
    for ci in range(NCORES):
        m = dict(shared)
        m["x"] = np.ascontiguousarray(x[BPC * ci:BPC * (ci + 1)])
        in_maps.append(m)
    return in_maps


def run(inputs, trace=False, tmpdir=None):
    if "nc" not in _CACHE:
        _CACHE["nc"] = build()
    nc = _CACHE["nc"]
    in_maps = prep_inputs(**inputs)
    kwargs = {}
    if trace:
        kwargs["trace"] = True
    if tmpdir:
        kwargs["tmpdir"] = tmpdir
    res = run_bass_kernel_spmd(nc, in_maps, core_ids=list(range(NCORES)), **kwargs)
    out = np.concatenate([r["out"] for r in res.results], axis=0)
    return out.reshape(B, C, HH, WW), res


def kernel(**inputs):
    return run(inputs)[0]
